# revision 10
# baseline (speedup 1.0000x reference)
"""Causal self-attention kernel for 8 trn2 NeuronCores.

Sharding: 4 batches x 2 head-groups (8 heads each). Core c handles
batch c//2, heads (c%2)*8 .. (c%2)*8+8. Each core computes qkv for its
head-group, causal attention, and a partial projection; the host sums
the two head-group partials per batch and adds b_proj.

v2 structure (vs v1 hp-major): ic-outer attention so the output
projection for each 512-token chunk is PSUM-accumulated over all 4
head-pairs right after the chunk completes, overlapping the next
chunk's attention. V GEMM groups g>=1 and the previous chunk's proj
are emitted inside the attention stream as PE filler (the attention
inner loop is ScalarE(exp)-bound). All SBUF operands are fp16 (PSUM
stays f32): same PE rate as f32r but permits N<256 matmuls at full
rate, 2-4x DVE modes, and half the DMA/SBUF footprint. Diagonal
blocks are column-trimmed: scores/exp/PV only touch cols >= 128*tdx
of the 512-wide chunk; a single shared 128x128 triangular mask zeroes
the boundary block (which also covers the stale-PSUM region that the
trimmed exp never wrote).

Per-core engine budget (est): PE ~190us (V 27 + QK 55 + scores ~58 +
PV ~58 + proj 27 - trim), ScalarE ~155us (exp + qk evac), DVE ~60us,
Pool ~50us.
"""

import sys
import os

for _p in ("/opt/trn_rl_repo", "/root/.axon_site/_ro/trn_rl_repo"):
    if os.path.isdir(_p) and _p not in sys.path:
        sys.path.insert(0, _p)

import numpy as np
import concourse.bass as bass  # noqa: F401
import concourse.mybir as mybir
import concourse.tile as tile
from concourse import bacc, bass_utils

F32 = mybir.dt.float32
F16 = mybir.dt.float16
ActF = mybir.ActivationFunctionType

B, S, D, H = 4, 2048, 1024, 16
NH = 8          # heads per core
HPAIRS = NH // 2
KT = D // 128   # 8 k-tiles over D
IC = S // 512   # 4 query chunks
NT = S // 128   # 16 token tiles
N_CORES = 8

_nc_cache = {}


def build_nc():
    if "nc" in _nc_cache:
        return _nc_cache["nc"]
    nc = bacc.Bacc("TRN2", target_bir_lowering=False, debug=False,
                   num_devices=N_CORES)
    xT = nc.dram_tensor("xT", [D, S], F16, kind="ExternalInput").ap()
    Wq = nc.dram_tensor("Wq", [D, 512], F16, kind="ExternalInput").ap()
    Wk = nc.dram_tensor("Wk", [D, 512], F16, kind="ExternalInput").ap()
    Wv = nc.dram_tensor("Wv", [D, 512], F16, kind="ExternalInput").ap()
    bq = nc.dram_tensor("bq", [1, 512], F32, kind="ExternalInput").ap()
    bk = nc.dram_tensor("bk", [1, 512], F32, kind="ExternalInput").ap()
    bv = nc.dram_tensor("bv", [1, 512], F32, kind="ExternalInput").ap()
    Wp = nc.dram_tensor("Wp", [512, D], F16, kind="ExternalInput").ap()
    maskD = nc.dram_tensor("maskD", [128, 256], F16,
                           kind="ExternalInput").ap()
    out = nc.dram_tensor("out", [S, D], F16, kind="ExternalOutput").ap()

    with tile.TileContext(nc) as tc:
        with tc.tile_pool(name="pp", bufs=1) as pp:
            xtr = [pp.tile([128, S], F16, name=f"xtr{k}") for k in range(KT)]
            wv = [pp.tile([128, 512], F16, name=f"wv{k}") for k in range(KT)]
            # v in natural layout, 65-stride per head (64 v cols + ones col)
            v_sb = [pp.tile([128, 8 * 65], F16, name=f"vsb{t}")
                    for t in range(NT)]
            qt = [pp.tile([128, S], F16, name=f"qt{h}") for h in range(HPAIRS)]
            kt = [pp.tile([128, S], F16, name=f"kt{h}") for h in range(HPAIRS)]
            yT = [pp.tile([128, S], F16, name=f"yT{h}") for h in range(HPAIRS)]
            wp = [pp.tile([128, D], F16, name=f"wp{h}") for h in range(HPAIRS)]
            # mask2[p, h*128+c] = 1 if c >= p else 0 (same tri for both heads)
            mask2 = pp.tile([128, 2, 128], F16, name="mask2")
            bvb = pp.tile([128, 512], F32, name="bvb")
            bvr = pp.tile([1, 512], F32, name="bvr")
            ones8 = pp.tile([128, 8, 1], F16, name="ones8")
            wq9 = [pp.tile([128, 1], F32, name=f"wq9_{h}")
                   for h in range(HPAIRS)]
            wk9 = [pp.tile([128, 1], F32, name=f"wk9_{h}")
                   for h in range(HPAIRS)]

            # ---- DMA issue order: small consts, weights, x col-chunks ----
            nc.sync.dma_start(mask2, maskD.rearrange("p (h c) -> p h c", c=128))
            nc.sync.dma_start(bvr, bv)
            for h in range(HPAIRS):
                nc.sync.dma_start(wq9[h], bq[0:1, h * 128:(h + 1) * 128])
                nc.sync.dma_start(wk9[h], bk[0:1, h * 128:(h + 1) * 128])
            for k in range(KT):
                nc.sync.dma_start(wv[k], Wv[k * 128:(k + 1) * 128, :])
            for k in range(KT):
                nc.sync.dma_start(xtr[k][:, 0:512],
                                  xT[k * 128:(k + 1) * 128, 0:512])
            nc.gpsimd.partition_broadcast(bvb, bvr)
            nc.gpsimd.memset(ones8, 1.0)

            with tc.tile_pool(name="ps", bufs=3, space="PSUM") as pssp, \
                 tc.tile_pool(name="py", bufs=2, space="PSUM") as psyp, \
                 tc.tile_pool(name="hb", bufs=1) as hb:

                def emit_v_group(g):
                    """v for token tiles 4g..4g+3 -> v_sb (fp16, +ones col)."""
                    for half in range(2):
                        ps = pssp.tile([128, 1024], F32, tag="pss",
                                       name="psv")
                        for k in range(KT):
                            for r in range(2):
                                t = 4 * g + 2 * half + r
                                nc.tensor.matmul(
                                    ps[:, r * 512:(r + 1) * 512],
                                    xtr[k][:, t * 128:(t + 1) * 128],
                                    wv[k], start=(k == 0), stop=(k == KT - 1))
                        for r in range(2):
                            t = 4 * g + 2 * half + r
                            vv = v_sb[t].rearrange("p (h c) -> p h c", c=65)
                            nc.vector.tensor_add(
                                vv[:, :, 0:64],
                                ps[:, r * 512:(r + 1) * 512]
                                .rearrange("p (h c) -> p h c", c=64),
                                bvb.rearrange("p (h c) -> p h c", c=64))
                            nc.vector.tensor_copy(vv[:, :, 64:65], ones8)

                def fetch_w(hp):
                    # bufs=4: all four head-pairs' weights stay resident
                    # (half-outer QK rereads them after later fetches)
                    wq_, wk_ = [], []
                    for k in range(KT):
                        tq = hb.tile([128, 128], F16, tag=f"wq{k}", bufs=4,
                                     name="wqt")
                        nc.sync.dma_start(
                            tq, Wq[k * 128:(k + 1) * 128,
                                   hp * 128:(hp + 1) * 128])
                        wq_.append(tq)
                        tk = hb.tile([128, 128], F16, tag=f"wk{k}", bufs=4,
                                     name="wkt")
                        nc.sync.dma_start(
                            tk, Wk[k * 128:(k + 1) * 128,
                                   hp * 128:(hp + 1) * 128])
                        wk_.append(tk)
                    return wq_, wk_

                def emit_qk_half(hp, w, half):
                    wq_, wk_ = w
                    for dst, ws, w9 in ((qt, wq_, wq9), (kt, wk_, wk9)):
                        ps = pssp.tile([128, 1024], F32, tag="pss",
                                       name="psq")
                        for k in range(KT):
                            for sub in range(2):
                                ch = half * 2 + sub
                                nc.tensor.matmul(
                                    ps[:, sub * 512:(sub + 1) * 512],
                                    ws[k],
                                    xtr[k][:, ch * 512:(ch + 1) * 512],
                                    start=(k == 0), stop=(k == KT - 1))
                        nc.scalar.activation(
                            dst[hp][:, half * 1024:(half + 1) * 1024],
                            ps, ActF.Identity, bias=w9[hp])

                def emit_proj(ic):
                    """projection for token chunk ic (needs all yT chunks)."""
                    for tt in range(4 * ic, 4 * ic + 4):
                        ps = pssp.tile([128, 1024], F32, tag="pss",
                                       name="pso")
                        for k in range(HPAIRS):
                            for nch in range(2):
                                nc.tensor.matmul(
                                    ps[:, nch * 512:(nch + 1) * 512],
                                    yT[k][:, tt * 128:(tt + 1) * 128],
                                    wp[k][:, nch * 512:(nch + 1) * 512],
                                    start=(k == 0), stop=(k == HPAIRS - 1))
                        for nch in range(2):
                            ot = hb.tile([128, 512], F16, tag="ot", bufs=3,
                                         name="ot")
                            nc.vector.tensor_copy(
                                ot, ps[:, nch * 512:(nch + 1) * 512])
                            nc.sync.dma_start(
                                out[tt * 128:(tt + 1) * 128,
                                    nch * 512:(nch + 1) * 512], ot)

                # ---- phase A: V(0) + all QK ----
                # DMA order matters: queues are FIFO. Weight fetches for
                # hp 0/1 go out before the bulk x chunks; QK runs
                # half-outer so the first 16 GEMMs per hp only need x
                # cols 0:1024 while cols 1024:2048 stream in.
                ws = [fetch_w(0), fetch_w(1)]
                for c in range(1, S // 512):
                    cs = slice(c * 512, (c + 1) * 512)
                    for k in range(KT):
                        nc.sync.dma_start(xtr[k][:, cs],
                                          xT[k * 128:(k + 1) * 128, cs])
                ws.append(fetch_w(2))
                ws.append(fetch_w(3))
                for h in range(HPAIRS):
                    nc.sync.dma_start(wp[h], Wp[h * 128:(h + 1) * 128, :])
                emit_v_group(0)
                for half in range(2):
                    for hp in range(HPAIRS):
                        emit_qk_half(hp, ws[hp], half)

                # ---- phase B: attention ic-outer, proj/V as PE filler ----
                for ic in range(IC):
                    nj = 4 * ic + 4
                    for hp in range(HPAIRS):
                        psA = psyp.tile([65, 512], F32, tag="psy",
                                        name="psyA")
                        psB = psyp.tile([65, 512], F32, tag="psy",
                                        name="psyB")
                        for jt in range(nj):
                            tdx = jt - 4 * ic
                            off = 128 * tdx if tdx > 0 else 0
                            ps = pssp.tile([128, 1024], F32, tag="pss",
                                           name="pscr")
                            nc.tensor.matmul(
                                ps[:, off:512],
                                kt[hp][0:64, jt * 128:(jt + 1) * 128],
                                qt[hp][0:64, ic * 512 + off:(ic + 1) * 512],
                                start=True, stop=True, tile_position=(0, 0))
                            nc.tensor.matmul(
                                ps[:, 512 + off:1024],
                                kt[hp][64:128, jt * 128:(jt + 1) * 128],
                                qt[hp][64:128, ic * 512 + off:(ic + 1) * 512],
                                start=True, stop=True, tile_position=(64, 0))
                            et = hb.tile([128, 1024], F16, tag="et", bufs=3,
                                         name="et")
                            if off == 0:
                                nc.scalar.activation(et, ps, ActF.Exp,
                                                     scale=0.125)
                            else:
                                e3 = et.rearrange("p (h c) -> p h c",
                                                  c=512)[:, :, off:512]
                                p3 = ps.rearrange("p (h c) -> p h c",
                                                  c=512)[:, :, off:512]
                                nc.scalar.activation(e3, p3, ActF.Exp,
                                                     scale=0.125)
                            if tdx >= 0:
                                e128 = et.rearrange(
                                    "p (h c) -> p h c",
                                    c=512)[:, :, off:off + 128]
                                nc.vector.tensor_mul(e128, e128, mask2)
                            for head, pY in ((0, psA), (1, psB)):
                                vsl = v_sb[jt][:, (2 * hp + head) * 65:
                                               (2 * hp + head) * 65 + 65]
                                nc.tensor.matmul(
                                    pY[:, off:512], vsl,
                                    et[:, head * 512 + off:
                                       head * 512 + 512],
                                    start=(jt == 0), stop=(jt == nj - 1),
                                    skip_group_check=True)
                        # evac + normalize chunk ic of this head pair
                        sl = slice(ic * 512, (ic + 1) * 512)
                        zc = hb.tile([1, 1024], F16, tag="zc", bufs=2,
                                     name="zc")
                        for head, pY in ((0, psA), (1, psB)):
                            t65 = hb.tile([65, 512], F16, tag="t65", bufs=2,
                                          name="t65")
                            nc.vector.tensor_copy(t65, pY)
                            nc.sync.dma_start(
                                yT[hp][head * 64:(head + 1) * 64, sl],
                                t65[0:64, :])
                            nc.sync.dma_start(
                                zc[0:1, head * 512:(head + 1) * 512],
                                t65[64:65, :])
                        # reciprocal at full lane width: scatter the 1024 Z
                        # values over 128 partitions (1-lane recip is ~6.5us)
                        zs = hb.tile([128, 8], F16, tag="zs", bufs=2,
                                     name="zs")
                        nc.sync.dma_start(zs, zc)
                        with nc.allow_low_precision("fp16 1/Z, 5e-4 rel"):
                            nc.vector.reciprocal(zs, zs)
                        nc.sync.dma_start(zc, zs)
                        bcf = hb.tile([128, 512], F16, tag="bcf", bufs=2,
                                      name="bcf")
                        nc.gpsimd.partition_broadcast(bcf, zc[0:1, 512:1024])
                        nc.gpsimd.partition_broadcast(bcf[0:64, :],
                                                      zc[0:1, 0:512])
                        nc.vector.tensor_mul(yT[hp][:, sl], yT[hp][:, sl],
                                             bcf)
                        # PE filler between head pairs: previous chunk's
                        # proj, then the next V group
                        if hp == 0 and ic > 0:
                            emit_proj(ic - 1)
                        if hp == 1 and ic < IC - 1:
                            emit_v_group(ic + 1)
                emit_proj(IC - 1)
    nc.finalize()
    _nc_cache["nc"] = nc
    return nc


def make_in_maps(x, W_attn, b_attn, W_proj):
    """Build per-core input dicts from full inputs."""
    tri = (np.arange(128)[None, :] >= np.arange(128)[:, None])
    mask2 = np.tile(tri.astype(np.float16), (1, 2))
    xT16 = [np.ascontiguousarray(x[b].T).astype(np.float16)
            for b in range(B)]
    in_maps = []
    for c in range(N_CORES):
        b = c // 2
        g = c % 2
        cs = slice(g * 512, (g + 1) * 512)
        in_maps.append({
            "xT": xT16[b],
            "Wq": np.ascontiguousarray(
                W_attn[:, 0:D][:, cs]).astype(np.float16),
            "Wk": np.ascontiguousarray(
                W_attn[:, D:2 * D][:, cs]).astype(np.float16),
            "Wv": np.ascontiguousarray(
                W_attn[:, 2 * D:3 * D][:, cs]).astype(np.float16),
            "bq": np.ascontiguousarray(
                b_attn[0:D][cs][None, :]).astype(np.float32),
            "bk": np.ascontiguousarray(
                b_attn[D:2 * D][cs][None, :]).astype(np.float32),
            "bv": np.ascontiguousarray(
                b_attn[2 * D:3 * D][cs][None, :]).astype(np.float32),
            "Wp": np.ascontiguousarray(W_proj[cs, :]).astype(np.float16),
            "maskD": mask2,
        })
    return in_maps


def kernel(x, W_attn, b_attn, W_proj, b_proj, trace=False):
    x = np.asarray(x, dtype=np.float32)
    W_attn = np.asarray(W_attn, dtype=np.float32)
    b_attn = np.asarray(b_attn, dtype=np.float32)
    W_proj = np.asarray(W_proj, dtype=np.float32)
    b_proj = np.asarray(b_proj, dtype=np.float32)
    nc = build_nc()
    in_maps = make_in_maps(x, W_attn, b_attn, W_proj)
    res = bass_utils.run_bass_kernel_spmd(
        nc, in_maps, core_ids=list(range(N_CORES)), trace=trace)
    outp = np.empty((B, S, D), dtype=np.float32)
    for b in range(B):
        outp[b] = (res.results[2 * b]["out"].astype(np.float32)
                   + res.results[2 * b + 1]["out"].astype(np.float32)
                   + b_proj[None, :])
    if trace:
        return outp, res
    return outp
